# revision 78
# baseline (speedup 1.0000x reference)
"""AttentionAugmentedConv2d Trainium2 Bass kernel (v2, interleaved).

Data-parallel over batch: 8 samples -> 8 NeuronCores, one sample per core.
Self-contained: hardcodes all shapes; builds derived constant inputs on host.

v2 changes vs v1 (240.7us -> 189.4us HW, rel l2 err 6.3e-3):
  - All matmuls in bf16 (walrus requires fp32/fp32r operands to pair, so the
    fp32r-moving/bf16-stationary mix is illegal; PSUM accumulation stays
    fp32). LDWEIGHTS drops 192ns -> 85ns (FWL) and hides behind matmuls.
  - Attention pairs of heads col-tiled: two M=64 matmuls at tile_position
    (0,0)/(0,64) run concurrently in the PE array -> half the att time.
  - Global interleave: the attention head pipeline (logits -> exp -> att)
    starts right after the q0/k0 convs; remaining conv blocks, vT transposes
    and rel-table work are emitted as PE "filler" between logits u-groups so
    the PE keeps busy while ScalarE chews the 8.4M-element exp stream.
    Dependencies are enforced by pull-based forcing (force_block/do_rel/
    do_vt) so emission order can never read unwritten tiles.
  - DMA: only sync(SP)/scalar(ACT) have HWDGE queues and each dma_start
    costs ~0.6us of sequencer DIRECT2D time, so the skew bounce writes both
    m'-halves in one descriptor, x arrives host-padded (no memset/copy),
    weights all prefetch before the skew traffic, and late constants load
    behind the pre-phase.
  - Tail: last pair's PSUM evacuation, softmax normalize, and the 1x1 conv
    pipeline per m'-half.
"""

import sys

sys.path.insert(0, "/opt/trn_rl_repo")

import numpy as np

import concourse.bass as bass
import concourse.tile as tile
from concourse import bacc, mybir
from concourse.bass_types import AP

F32 = mybir.dt.float32
F32R = mybir.dt.float32r
BF16 = mybir.dt.bfloat16

B, CIN, COUT, DK, DV, NH, H, W = 8, 256, 512, 256, 256, 8, 32, 32
DKH = DK // NH  # 32
HW = H * W  # 1024
SCALE = DKH ** -0.5
N_CORES = 8
FILL_PER_U = 7  # conv MMs of filler per logits u-group

_cached = {}


def _r(ap):
    return ap.bitcast(F32R)


def build_bass():
    nc = bacc.Bacc("TRN2", target_bir_lowering=False, debug=False,
                   num_devices=N_CORES)

    x_d = nc.dram_tensor("x", [2, 128, 34 * 34], BF16,
                        kind="ExternalInput").ap()
    # conv weights: (cout_tile, ci_half, ci_local, tap, co_local);
    # qkv tiles 0-5 (q0 q1 k0 k1 v0 v1), conv tiles 6-7
    wq_d = nc.dram_tensor("wqkvT", [6, 2, 128, 9, 128], BF16,
                          kind="ExternalInput").ap()
    wc_d = nc.dram_tensor("wconvT", [2, 2, 128, 9, 128], BF16,
                          kind="ExternalInput").ap()
    wattT_d = nc.dram_tensor("wattT", [2, 128, 256], BF16,
                             kind="ExternalInput").ap()
    bcol_d = nc.dram_tensor("bias_cols", [128, 10], F32,
                            kind="ExternalInput").ap()
    krc_d = nc.dram_tensor("krcb", [2, 4, 128, 128], BF16,
                           kind="ExternalInput").ap()
    ltc_d = nc.dram_tensor("ltc", [96, HW], BF16, kind="ExternalInput").ap()
    rtz_d = nc.dram_tensor("rtz", [32, HW], BF16,
                           kind="ExternalInput").ap()
    i128_d = nc.dram_tensor("I128", [128, 128], BF16,
                            kind="ExternalInput").ap()
    vinit_d = nc.dram_tensor("vinit", [576], BF16, kind="ExternalInput").ap()
    out_d = nc.dram_tensor("out", [COUT, HW], F32, kind="ExternalOutput").ap()

    with tile.TileContext(nc) as tc:
        _build(nc, tc, x_d, wq_d, wc_d, wattT_d, bcol_d, krc_d,
               ltc_d, rtz_d, i128_d, vinit_d, out_d)
    nc.compile()
    return nc


def _build(nc, tc, x_d, wq_d, wc_d, wattT_d, bcol_d, krc_d,
           ltc_d, rtz_d, i128_d, vinit_d, out_d):
    from contextlib import ExitStack

    Exp = mybir.ActivationFunctionType.Exp
    ADD = mybir.AluOpType.add
    MULT = mybir.AluOpType.mult

    ctx = ExitStack()
    with ctx:
        persist = ctx.enter_context(tc.tile_pool(name="persist", bufs=1))
        wpool = ctx.enter_context(tc.tile_pool(name="wpool", bufs=16))
        xfer = ctx.enter_context(tc.tile_pool(name="xfer", bufs=2))
        e_pool = ctx.enter_context(tc.tile_pool(name="epool", bufs=10))
        rel_pool = ctx.enter_context(tc.tile_pool(name="relsb", bufs=4))
        psum = ctx.enter_context(
            tc.tile_pool(name="psum", bufs=2, space="PSUM"))
        dram_pool = ctx.enter_context(
            tc.tile_pool(name="dramp", bufs=1, space="DRAM"))

        # ---------- persistent SBUF ----------
        xpad = [persist.tile([128, 34 * 34], BF16, tag=f"xpad{i}",
                             name=f"xpad{i}") for i in range(2)]
        q_st = [persist.tile([128, HW], BF16, tag=f"qst{i}", name=f"qst{i}")
                for i in range(2)]
        k_st = [persist.tile([128, HW], BF16, tag=f"kst{i}", name=f"kst{i}")
                for i in range(2)]
        v_st = [persist.tile([128, HW], BF16, tag=f"vst{i}", name=f"vst{i}")
                for i in range(2)]
        vT_t = [persist.tile([128, 576], BF16, tag=f"vT{u}", name=f"vT{u}")
                for u in range(8)]
        i128 = persist.tile([128, 128], BF16, tag="i128", name="i128")
        wattT_sb = [persist.tile([128, 256], BF16, tag=f"watt{i}",
                                 name=f"watt{i}") for i in range(2)]
        bcol = persist.tile([128, 10], F32, tag="bcol", name="bcol")
        # key_rel blocks: (128, 128) zero-padded, key_rel^T at rows 32a
        krc = [[persist.tile([128, 128], BF16, tag=f"krc{t}_{a}",
                             name=f"krc{t}_{a}") for a in range(4)]
               for t in range(2)]
        att_un = [persist.tile([128, HW], F32, tag=f"attun{g}",
                               name=f"attun{g}") for g in range(2)]
        smap = [persist.tile([128, HW], F32, tag=f"smap{g}", name=f"smap{g}")
                for g in range(2)]
        rmap = [persist.tile([128, HW], F32, tag=f"rmap{g}", name=f"rmap{g}")
                for g in range(2)]
        attn = [persist.tile([128, HW], BF16, tag=f"attn{g}", name=f"attn{g}")
                for g in range(2)]
        rscr = persist.tile([128, HW], F32, tag="rscr", name="rscr")

        # lt ring: [k_h; I_w; I_h; 0] bf16; rt ring: [q_h; A_w; A_h'; 0] f32
        lt_r = [persist.tile([128, HW], BF16, tag=f"lt{s}", name=f"lt{s}")
                for s in range(4)]
        rt_r = [persist.tile([128, HW], BF16, tag=f"rt{s}", name=f"rt{s}")
                for s in range(4)]
        skw = [[dram_pool.tile([94 * HW], BF16, tag=f"skw{h}_{t}",
                               name=f"skw{h}_{t}") for t in range(2)]
               for h in range(NH)]

        # ---------- startup: x (host-padded) + first conv weights ----------
        nc.sync.dma_start(xpad[0][:], x_d[0])
        nc.scalar.dma_start(xpad[1][:], x_d[1])
        wq0 = []
        for c in range(2):
            wt = wpool.tile([128, 9 * 128], BF16, tag="w", name=f"w_q0{c}")
            eng = nc.sync if c == 0 else nc.scalar
            eng.dma_start(wt[:], wq_d[0][c].rearrange("p t co -> p (t co)"))
            wq0.append(wt)
        nc.scalar.dma_start(bcol[:], bcol_d[:])

        # ---------- early constants (gpsimd SWDGE: engine is idle at
        # startup, and this keeps the scalar HWDGE queue clear so the rel
        # copies + skew writes fire as soon as the rel matmuls finish) ----
        for t in range(2):
            for a in range(4):
                nc.gpsimd.dma_start(krc[t][a][:], krc_d[t][a])
        for s in range(4):
            nc.gpsimd.dma_start(lt_r[s][32:128, :], ltc_d[:])
            nc.gpsimd.dma_start(rt_r[s][96:128, :], rtz_d[:])
        nc.gpsimd.dma_start(i128[:], i128_d[:])

        def xwin(cih, r0, nr, dy, dx):
            v = xpad[cih][:].rearrange("p (a b) -> p a b", a=34)
            return v[:, dy + r0: dy + r0 + nr, dx:dx + 32]

        # ---------- conv blocks as resumable filler jobs ----------
        class Block:
            def __init__(self, w_src, epi, name):
                self.w_src, self.epi, self.name = w_src, epi, name
                self.ws = None
                self.st = 0
                self.i = 0
                self.cps = None
                self.done = False

            def prefetch(self):
                if self.ws is None:
                    self.ws = []
                    for c in range(2):
                        wt = wpool.tile([128, 9 * 128], BF16, tag="w",
                                        name=f"w_{self.name}{c}")
                        nc.sync.dma_start(
                            wt[:],
                            self.w_src[c].rearrange("p t co -> p (t co)"))
                        self.ws.append(wt)

            def emit(self, n):
                self.prefetch()
                emitted = 0
                while n > 0 and not self.done:
                    if self.i == 0:
                        self.cps = psum.tile([128, 512], F32, tag="cps",
                                             name=f"c_{self.name}{self.st}")
                    t, c = self.i // 2, self.i % 2
                    dy, dx = t // 3, t % 3
                    nc.tensor.matmul(
                        self.cps[:], self.ws[c][:, 128 * t:128 * (t + 1)],
                        xwin(c, 16 * self.st, 16, dy, dx),
                        start=(self.i == 0), stop=(self.i == 17))
                    self.i += 1
                    emitted += 1
                    n -= 1
                    if self.i == 18:
                        self.epi(self.st, self.cps)
                        self.i = 0
                        self.st += 1
                        self.done = self.st == 2
                return emitted

        def qkv_epi(cc):
            def epi(st, cps):
                b = bcol[:, cc:cc + 1]
                if cc < 2:
                    qv = q_st[cc][:].rearrange("p (c r) -> p r c", r=32)
                    nc.vector.tensor_scalar(
                        qv[:, 16 * st:16 * (st + 1), :],
                        cps[:].rearrange("p (r c) -> p r c", r=16),
                        b, SCALE, ADD, MULT)
                else:
                    dst = (k_st[cc - 2] if cc < 4 else v_st[cc - 4])
                    nc.vector.tensor_scalar(
                        dst[:, 512 * st:512 * (st + 1)], cps[:], b, None, ADD)
            return epi

        def xo_epi(cc):
            def epi(st, cps):
                osb = rel_pool.tile([128, 512], F32, tag="osb", name="osb")
                nc.vector.tensor_scalar(
                    osb[:], cps[:], bcol[:, 6 + cc:7 + cc], None, ADD)
                nc.sync.dma_start(
                    out_d[128 * cc:128 * (cc + 1),
                          512 * st:512 * (st + 1)], osb[:])
            return epi

        blocks = {
            "q0": Block(wq_d[0], qkv_epi(0), "q0"),  # ws pre-loaded above
        }
        blocks["q0"].ws = wq0
        blocks.update({
            "k0": Block(wq_d[2], qkv_epi(2), "k0"),
            "v0": Block(wq_d[4], qkv_epi(4), "v0"),
            "q1": Block(wq_d[1], qkv_epi(1), "q1"),
            "k1": Block(wq_d[3], qkv_epi(3), "k1"),
            "v1": Block(wq_d[5], qkv_epi(5), "v1"),
            "xo0": Block(wc_d[0], xo_epi(0), "xo0"),
            "xo1": Block(wc_d[1], xo_epi(1), "xo1"),
        })

        # ---------- rel tables + skew bounce (per (h, tab) step) ----------
        def rel_step(h, tab, copy_eng):
            qt, j4 = h // 4, h % 4
            rsb = rel_pool.tile([64, HW], BF16, tag="rsb", name="rsb")
            if h < 2:
                rl = psum.tile([128, HW], F32, tag="lps", name=f"rl{h}{tab}")
                views = [rl[:, 0:512], rl[:, 512:HW]]
            else:
                views = [psum.tile([128, 512], F32, tag="cps",
                                   name=f"r{h}{tab}{st}")[:]
                         for st in range(2)]
            for st in range(2):
                if tab == 0:
                    qrhs = q_st[qt][:, 512 * st:512 * (st + 1)]
                else:
                    qrhs = q_st[qt][:].rearrange(
                        "p (c r) -> p r c", r=32)[:, 16 * st:16 * (st + 1), :]
                rps = views[st]
                nc.tensor.matmul(rps, krc[tab][j4][:], qrhs,
                                 start=True, stop=True)
                if copy_eng is nc.scalar:
                    nc.scalar.activation(
                        rsb[0:63, 512 * st:512 * (st + 1)], rps[0:63, :],
                        mybir.ActivationFunctionType.Copy)
                else:
                    nc.vector.tensor_copy(
                        rsb[0:63, 512 * st:512 * (st + 1)], rps[0:63, :])
            dst = AP(skw[h][tab].tensor, 0, [[HW, 63], [1056, 32], [1, 32]])
            deng = nc.scalar if (h < 4 and tab == 1) else nc.sync
            deng.dma_start(
                dst, rsb[0:63, :].rearrange("p (a b) -> p a b", a=32))

        # ---------- v -> vT transpose steps ----------
        def vt_step(u, half):
            vps = psum.tile([128, 512], BF16, tag="cps", name=f"v{u}{half}")
            nc.tensor.transpose(
                vps[:, 0:128], v_st[half][:, 128 * u:128 * (u + 1)], i128[:])
            dst = vT_t[u][:].rearrange("p (h j) -> p h j", h=9)
            nc.vector.tensor_copy(
                dst[:, 4 * half:4 * (half + 1), 0:32],
                vps[:, 0:128].rearrange("p (h d) -> p h d", h=4))

        # ---------- filler queue (with pull-based forcing) ----------
        rel_done = set()
        vt_done = set()

        def do_rel(h, t):
            if (h, t) not in rel_done:
                rel_done.add((h, t))
                rel_step(h, t, nc.scalar if h < 2 else nc.vector)
                return 2
            return 0

        def do_vt(u, half):
            if (u, half) not in vt_done:
                vt_done.add((u, half))
                vt_step(u, half)
                return 1
            return 0

        fillers = (
            [("blk", "v0")]
            + [("vt", u, 0) for u in range(8)]
            + [("blk", "q1"), ("blk", "k1"), ("blk", "v1")]
            + [("vt", u, 1) for u in range(8)]
            + [("rel", h, t) for h in (4, 5, 6, 7) for t in range(2)]
            + [("blk", "xo0"), ("blk", "xo1")]
        )

        def emit_filler(budget):
            while budget > 0 and fillers:
                item = fillers[0]
                if item[0] == "blk":
                    blk = blocks[item[1]]
                    if blk.done:
                        fillers.pop(0)
                        continue
                    got = blk.emit(budget)
                    budget -= got
                    if blk.done:
                        fillers.pop(0)
                elif item[0] == "rel":
                    budget -= do_rel(*item[1:])
                    fillers.pop(0)
                else:
                    budget -= 2 * do_vt(*item[1:])
                    fillers.pop(0)

        def force_block(name):
            blk = blocks[name]
            if not blk.done:
                blk.emit(100)

        def ensure_stage_deps(h):
            if h >= 4:
                force_block("q1")
                force_block("k1")
            for t in range(2):
                do_rel(h, t)

        def ensure_vt(half):
            force_block("v1" if half else "v0")
            for u in range(8):
                do_vt(u, half)

        # prefetch ALL weight tiles up front: the sync queue is free now and
        # gets congested with skew-bounce traffic once the rel phase starts
        blocks["k0"].prefetch()
        blocks["v0"].prefetch()
        for name in ("q1", "k1", "v1", "xo0", "xo1"):
            blocks[name].prefetch()

        # ---------- pre-phase: q0 -> rel h0/h1 -> k0 -> staging ----------
        blocks["q0"].emit(36)

        # ---------- head staging ----------
        def stage_head(h):
            ensure_stage_deps(h)
            ceng = nc.vector
            meng = nc.vector if h < 4 else nc.gpsimd
            reng = nc.sync
            slot = h % 4
            lt, rtt = lt_r[slot], rt_r[slot]
            qt, j4 = h // 4, h % 4
            ceng.tensor_copy(lt[0:32, :],
                             k_st[qt][32 * j4:32 * j4 + 32, :])
            ceng.tensor_copy(rtt[0:32, :],
                             q_st[qt][32 * j4:32 * j4 + 32, :])
            skr = AP(skw[h][0].tensor, 31 * HW,
                     [[HW, 32], [32, 32], [1, 32]])
            reng.dma_start(
                rtt[32:64, :].rearrange("p (a b) -> p a b", a=32), skr)
            ah = xfer.tile([32, HW], BF16, tag="ah", name="ah", bufs=2)
            skr1 = AP(skw[h][1].tensor, 31 * HW,
                      [[HW, 32], [32, 32], [1, 32]])
            reng.dma_start(
                ah[:].rearrange("p (a b) -> p a b", a=32), skr1)
            meng.tensor_copy(
                rtt[64:96, :].rearrange("p (c r) -> p c r", c=32),
                ah[:].rearrange("p (r c) -> p c r", c=32))

        # rel h0/h1 right after q0 (latency chain, no filler spacing); the
        # k0 conv then overlaps the skew bounce DRAM round-trip
        for h in range(2):
            for t in range(2):
                do_rel(h, t)
        blocks["k0"].emit(36)
        for h in range(2):
            stage_head(h)

        # ---------- late constants: needed from first att / tail ----------
        for u in range(8):
            nc.sync.dma_start(
                vT_t[u][:], AP(vinit_d.tensor, 0, [[0, 128], [1, 576]]))
        for i in range(2):
            nc.sync.dma_start(wattT_sb[i][:], wattT_d[i])

        # ---------- 1x1 conv on one m'-half (both cout tiles) ----------
        def final_conv(st):
            for ct in range(2):
                ops = psum.tile([128, 512], F32, tag="ap", name=f"o{ct}{st}")
                for kc in range(2):
                    nc.tensor.matmul(
                        ops[:],
                        wattT_sb[kc][:, 128 * ct:128 * (ct + 1)],
                        attn[kc][:, 512 * st:512 * (st + 1)],
                        start=(kc == 0), stop=(kc == 1))
                osb = rel_pool.tile([128, 512], F32, tag="osb", name="osb")
                nc.scalar.activation(osb[:], ops[:],
                                     mybir.ActivationFunctionType.Copy)
                eng = nc.sync if ct == 0 else nc.scalar
                eng.dma_start(
                    out_d[256 + 128 * ct:256 + 128 * (ct + 1),
                          512 * st:512 * (st + 1)],
                    osb[:])

        # ---------- head phase: flat (pair, u) stream, atts lag LAG groups
        # behind logits so the next pair's logits cross the boundary ahead
        # of the previous pair's PSUM evacuation (keeps ScalarE's exp
        # stream fed through pair transitions) ----------
        LAG = 3
        es = {}
        aps = {}

        def emit_logits(p, u):
            for j in range(2):
                s = (2 * p + j) % 4
                lt, rtt = lt_r[s], rt_r[s]
                lps = psum.tile([128, HW], F32, tag="lps",
                                name=f"l{p}_{u}_{j}")
                for mh in range(2):
                    nc.tensor.matmul(
                        lps[:, 512 * mh:512 * (mh + 1)],
                        lt[:, 128 * u:128 * (u + 1)],
                        rtt[:, 512 * mh:512 * (mh + 1)],
                        start=True, stop=True)
                e = e_pool.tile([128, HW], BF16, tag="E", name=f"e{p}{u}{j}")
                nc.scalar.activation(e[:], lps[:], Exp)
                es[(p, u, j)] = e

        def pair_end(p):
            # evacuate att + sums via 32x32 block transposes, one m'-half
            # at a time so the last pair's normalize + 1x1 conv pipeline
            h0, h1 = 2 * p, 2 * p + 1
            g = p // 2
            po0, po1 = 32 * (h0 % 4), 32 * (h1 % 4)
            ap0, ap1 = aps.pop(p)
            for mh, apx in ((0, ap0), (1, ap1)):
                c0, c1 = 512 * mh, 512 * (mh + 1)
                nc.vector.transpose(att_un[g][po0:po0 + 32, c0:c1],
                                    apx[0:32, :])
                nc.vector.transpose(smap[g][po0:po0 + 32, c0:c1],
                                    apx[32:64, :])
                nc.vector.transpose(att_un[g][po1:po1 + 32, c0:c1],
                                    apx[64:96, :])
                nc.vector.transpose(smap[g][po1:po1 + 32, c0:c1],
                                    apx[96:128, :])
                if p % 2 == 1:
                    nc.vector.reciprocal_approx_fast(
                        rmap[g][:, c0:c1], smap[g][:, c0:c1])
                    nc.vector.tensor_tensor(
                        attn[g][:, c0:c1], att_un[g][:, c0:c1],
                        rmap[g][:, c0:c1], MULT)
                    if p == 3:
                        final_conv(mh)

        def emit_att(p, u):
            h0, h1 = 2 * p, 2 * p + 1
            if u == 0:
                force_block("v0" if h0 < 4 else "v1")
                aps[p] = (
                    psum.tile([128, 512], F32, tag="ap", name=f"ap0_{p}"),
                    psum.tile([128, 512], F32, tag="ap", name=f"ap1_{p}"))
            ap0, ap1 = aps[p]
            do_vt(u, h0 // 4)
            e0, e1 = es.pop((p, u, 0)), es.pop((p, u, 1))
            for mh, apx in ((0, ap0), (1, ap1)):
                nc.tensor.matmul(
                    apx[0:64, :],
                    vT_t[u][:, 64 * h0:64 * h0 + 64],
                    e0[:, 512 * mh:512 * (mh + 1)],
                    start=(u == 0), stop=(u == 7),
                    tile_position=(0, 0), skip_group_check=True)
                nc.tensor.matmul(
                    apx[64:128, :],
                    vT_t[u][:, 64 * h1:64 * h1 + 64],
                    e1[:, 512 * mh:512 * (mh + 1)],
                    start=(u == 0), stop=(u == 7),
                    tile_position=(0, 64), skip_group_check=True)
            if u == 7:
                pair_end(p)

        groups = [(p, u) for p in range(4) for u in range(8)]
        for i in range(len(groups) + LAG):
            if i < len(groups):
                p, u = groups[i]
                emit_logits(p, u)
                if u == 2 and p < 3:
                    stage_head(2 * p + 2)
                    stage_head(2 * p + 3)
            emit_filler(FILL_PER_U)
            if i >= LAG:
                emit_att(*groups[i - LAG])

        # drain any remaining filler work
        emit_filler(10 ** 6)


def _host_inputs(x, w_conv, b_conv, w_qkv, b_qkv, w_att, b_att,
                 key_rel_w, key_rel_h):
    """Build per-core input maps (host-side layout prep only)."""
    import ml_dtypes
    bf16 = ml_dtypes.bfloat16
    x = np.asarray(x, dtype=np.float32)

    def wT(w, nt):
        # (co, ci, 3, 3) -> (cout_tile, ci_half, ci_local, tap, co_local)
        w = np.asarray(w, dtype=np.float32).reshape(nt, 128, 2, 128, 9)
        return np.ascontiguousarray(w.transpose(0, 2, 3, 4, 1)).astype(bf16)

    wqkvT = wT(w_qkv, 6)
    wconvT = wT(w_conv, 2)
    wattT = np.ascontiguousarray(
        np.asarray(w_att, dtype=np.float32)[:, :, 0, 0].T.reshape(
            2, 128, 256)).astype(bf16)
    bias_cols = np.zeros((128, 10), np.float32)
    bias_cols[:, 0:6] = np.asarray(b_qkv, np.float32).reshape(6, 128).T
    bias_cols[:, 6:8] = np.asarray(b_conv, np.float32).reshape(2, 128).T
    bias_cols[:, 8:10] = np.asarray(b_att, np.float32).reshape(2, 128).T
    n = np.arange(HW)
    ltc = np.zeros((96, HW), np.float32)
    ltc[0:32] = np.arange(32)[:, None] == (n % 32)[None, :]   # I_w
    ltc[32:64] = np.arange(32)[:, None] == (n // 32)[None, :]  # I_h
    I128 = np.eye(128, dtype=np.float32).astype(bf16)
    vinit = np.tile(np.concatenate([
        np.zeros(32, np.float32), np.ones(32, np.float32)]), 9).astype(bf16)
    krcb = np.zeros((2, 4, 128, 128), np.float32)
    for t, kr in ((0, key_rel_w), (1, key_rel_h)):
        krT = np.asarray(kr, np.float32).T  # (32, 63)
        for a in range(4):
            krcb[t, a, 32 * a:32 * (a + 1), 0:63] = krT
    shared = {
        "wqkvT": wqkvT, "wconvT": wconvT, "wattT": wattT,
        "bias_cols": bias_cols,
        "krcb": krcb.astype(bf16),
        "ltc": ltc.astype(bf16),
        "rtz": np.zeros((32, HW), bf16),
        "I128": I128,
        "vinit": vinit,
    }
    xp = np.zeros((B, 2, 128, 34, 34), np.float32)
    xp[:, :, :, 1:33, 1:33] = x.reshape(B, 2, 128, 32, 32)
    xp = xp.reshape(B, 2, 128, 34 * 34).astype(bf16)
    return [dict(shared, x=xp[i]) for i in range(B)]


def get_nc():
    if "nc" not in _cached:
        _cached["nc"] = build_bass()
    return _cached["nc"]


def kernel(x, w_conv, b_conv, w_qkv, b_qkv, w_att, b_att,
           key_rel_w, key_rel_h):
    from concourse.bass_utils import run_bass_kernel_spmd
    nc = get_nc()
    in_maps = _host_inputs(x, w_conv, b_conv, w_qkv, b_qkv, w_att, b_att,
                           key_rel_w, key_rel_h)
    res = run_bass_kernel_spmd(nc, in_maps, list(range(N_CORES)))
    out = np.stack([res.results[i]["out"].reshape(COUT, H, W)
                    for i in range(B)])
    # b_att is a linear output bias; applied host-side so the device
    # epilogue is a plain ScalarE copy off the critical DVE tail
    out[:, DV:] += np.asarray(b_att, np.float32).reshape(1, DV, 1, 1)
    return out


# revision 82
# speedup vs baseline: 1.0258x; 1.0258x over previous
"""AttentionAugmentedConv2d Trainium2 Bass kernel (v2, interleaved).

Data-parallel over batch: 8 samples -> 8 NeuronCores, one sample per core.
Self-contained: hardcodes all shapes; builds derived constant inputs on host.

v2 changes vs v1 (240.7us -> 189.4us HW, rel l2 err 6.3e-3):
  - All matmuls in bf16 (walrus requires fp32/fp32r operands to pair, so the
    fp32r-moving/bf16-stationary mix is illegal; PSUM accumulation stays
    fp32). LDWEIGHTS drops 192ns -> 85ns (FWL) and hides behind matmuls.
  - Attention pairs of heads col-tiled: two M=64 matmuls at tile_position
    (0,0)/(0,64) run concurrently in the PE array -> half the att time.
  - Global interleave: the attention head pipeline (logits -> exp -> att)
    starts right after the q0/k0 convs; remaining conv blocks, vT transposes
    and rel-table work are emitted as PE "filler" between logits u-groups so
    the PE keeps busy while ScalarE chews the 8.4M-element exp stream.
    Dependencies are enforced by pull-based forcing (force_block/do_rel/
    do_vt) so emission order can never read unwritten tiles.
  - DMA: only sync(SP)/scalar(ACT) have HWDGE queues and each dma_start
    costs ~0.6us of sequencer DIRECT2D time, so the skew bounce writes both
    m'-halves in one descriptor, x arrives host-padded (no memset/copy),
    weights all prefetch before the skew traffic, and late constants load
    behind the pre-phase.
  - Tail: last pair's PSUM evacuation, softmax normalize, and the 1x1 conv
    pipeline per m'-half.
"""

import sys

sys.path.insert(0, "/opt/trn_rl_repo")

import numpy as np

import concourse.bass as bass
import concourse.tile as tile
from concourse import bacc, mybir
from concourse.bass_types import AP

F32 = mybir.dt.float32
F32R = mybir.dt.float32r
BF16 = mybir.dt.bfloat16

B, CIN, COUT, DK, DV, NH, H, W = 8, 256, 512, 256, 256, 8, 32, 32
DKH = DK // NH  # 32
HW = H * W  # 1024
SCALE = DKH ** -0.5
N_CORES = 8
FILL_PER_U = 7  # conv MMs of filler per logits u-group

_cached = {}


def _r(ap):
    return ap.bitcast(F32R)


def build_bass():
    nc = bacc.Bacc("TRN2", target_bir_lowering=False, debug=False,
                   num_devices=N_CORES)

    x_d = nc.dram_tensor("x", [2, 128, 34 * 34], BF16,
                        kind="ExternalInput").ap()
    # conv weights: (cout_tile, ci_half, ci_local, tap, co_local);
    # qkv tiles 0-5 (q0 q1 k0 k1 v0 v1), conv tiles 6-7
    wq_d = nc.dram_tensor("wqkvT", [6, 2, 128, 9, 128], BF16,
                          kind="ExternalInput").ap()
    wc_d = nc.dram_tensor("wconvT", [2, 2, 128, 9, 128], BF16,
                          kind="ExternalInput").ap()
    wattT_d = nc.dram_tensor("wattT", [2, 128, 256], BF16,
                             kind="ExternalInput").ap()
    bcol_d = nc.dram_tensor("bias_cols", [128, 10], F32,
                            kind="ExternalInput").ap()
    krc_d = nc.dram_tensor("krcb", [2, 4, 128, 128], BF16,
                           kind="ExternalInput").ap()
    ltc_d = nc.dram_tensor("ltc", [96, HW], BF16, kind="ExternalInput").ap()
    rtz_d = nc.dram_tensor("rtz", [32, HW], BF16,
                           kind="ExternalInput").ap()
    i128_d = nc.dram_tensor("I128", [128, 128], BF16,
                            kind="ExternalInput").ap()
    vinit_d = nc.dram_tensor("vinit", [576], BF16, kind="ExternalInput").ap()
    out_d = nc.dram_tensor("out", [COUT, HW], F32, kind="ExternalOutput").ap()

    with tile.TileContext(nc) as tc:
        _build(nc, tc, x_d, wq_d, wc_d, wattT_d, bcol_d, krc_d,
               ltc_d, rtz_d, i128_d, vinit_d, out_d)
    nc.compile()
    return nc


def _build(nc, tc, x_d, wq_d, wc_d, wattT_d, bcol_d, krc_d,
           ltc_d, rtz_d, i128_d, vinit_d, out_d):
    from contextlib import ExitStack

    Exp = mybir.ActivationFunctionType.Exp
    ADD = mybir.AluOpType.add
    MULT = mybir.AluOpType.mult

    ctx = ExitStack()
    with ctx:
        persist = ctx.enter_context(tc.tile_pool(name="persist", bufs=1))
        wpool = ctx.enter_context(tc.tile_pool(name="wpool", bufs=16))
        xfer = ctx.enter_context(tc.tile_pool(name="xfer", bufs=2))
        e_pool = ctx.enter_context(tc.tile_pool(name="epool", bufs=10))
        rel_pool = ctx.enter_context(tc.tile_pool(name="relsb", bufs=4))
        psum = ctx.enter_context(
            tc.tile_pool(name="psum", bufs=2, space="PSUM"))
        dram_pool = ctx.enter_context(
            tc.tile_pool(name="dramp", bufs=1, space="DRAM"))

        # ---------- persistent SBUF ----------
        xpad = [persist.tile([128, 34 * 34], BF16, tag=f"xpad{i}",
                             name=f"xpad{i}") for i in range(2)]
        q_st = [persist.tile([128, HW], BF16, tag=f"qst{i}", name=f"qst{i}")
                for i in range(2)]
        k_st = [persist.tile([128, HW], BF16, tag=f"kst{i}", name=f"kst{i}")
                for i in range(2)]
        v_st = [persist.tile([128, HW], BF16, tag=f"vst{i}", name=f"vst{i}")
                for i in range(2)]
        vT_t = [persist.tile([128, 576], BF16, tag=f"vT{u}", name=f"vT{u}")
                for u in range(8)]
        i128 = persist.tile([128, 128], BF16, tag="i128", name="i128")
        wattT_sb = [persist.tile([128, 256], BF16, tag=f"watt{i}",
                                 name=f"watt{i}") for i in range(2)]
        bcol = persist.tile([128, 10], F32, tag="bcol", name="bcol")
        # key_rel blocks: (128, 128) zero-padded, key_rel^T at rows 32a
        krc = [[persist.tile([128, 128], BF16, tag=f"krc{t}_{a}",
                             name=f"krc{t}_{a}") for a in range(4)]
               for t in range(2)]
        att_un = [persist.tile([128, HW], F32, tag=f"attun{g}",
                               name=f"attun{g}") for g in range(2)]
        smap = [persist.tile([128, HW], F32, tag=f"smap{g}", name=f"smap{g}")
                for g in range(2)]
        rmap = [persist.tile([128, HW], F32, tag=f"rmap{g}", name=f"rmap{g}")
                for g in range(2)]
        attn = [persist.tile([128, HW], BF16, tag=f"attn{g}", name=f"attn{g}")
                for g in range(2)]
        rscr = persist.tile([128, HW], F32, tag="rscr", name="rscr")

        # lt ring: [k_h; I_w; I_h; 0] bf16; rt ring: [q_h; A_w; A_h'; 0] f32
        lt_r = [persist.tile([128, HW], BF16, tag=f"lt{s}", name=f"lt{s}")
                for s in range(4)]
        rt_r = [persist.tile([128, HW], BF16, tag=f"rt{s}", name=f"rt{s}")
                for s in range(4)]
        skw = [[dram_pool.tile([94 * HW], BF16, tag=f"skw{h}_{t}",
                               name=f"skw{h}_{t}") for t in range(2)]
               for h in range(NH)]

        # ---------- startup: x (host-padded) + first conv weights ----------
        nc.sync.dma_start(xpad[0][:], x_d[0])
        nc.scalar.dma_start(xpad[1][:], x_d[1])
        wq0 = []
        for c in range(2):
            wt = wpool.tile([128, 9 * 128], BF16, tag="w", name=f"w_q0{c}")
            eng = nc.sync if c == 0 else nc.scalar
            eng.dma_start(wt[:], wq_d[0][c].rearrange("p t co -> p (t co)"))
            wq0.append(wt)
        nc.scalar.dma_start(bcol[:], bcol_d[:])

        # ---------- early constants (gpsimd SWDGE: engine is idle at
        # startup, and this keeps the scalar HWDGE queue clear so the rel
        # copies + skew writes fire as soon as the rel matmuls finish) ----
        for t in range(2):
            for a in range(4):
                nc.gpsimd.dma_start(krc[t][a][:], krc_d[t][a])
        for s in range(4):
            nc.gpsimd.dma_start(lt_r[s][32:128, :], ltc_d[:])
            nc.gpsimd.dma_start(rt_r[s][96:128, :], rtz_d[:])
        nc.gpsimd.dma_start(i128[:], i128_d[:])

        def xwin(cih, r0, nr, dy, dx):
            v = xpad[cih][:].rearrange("p (a b) -> p a b", a=34)
            return v[:, dy + r0: dy + r0 + nr, dx:dx + 32]

        # ---------- conv blocks as resumable filler jobs ----------
        class Block:
            def __init__(self, w_src, epi, name):
                self.w_src, self.epi, self.name = w_src, epi, name
                self.ws = None
                self.st = 0
                self.i = 0
                self.cps = None
                self.done = False

            def prefetch(self):
                if self.ws is None:
                    self.ws = []
                    for c in range(2):
                        wt = wpool.tile([128, 9 * 128], BF16, tag="w",
                                        name=f"w_{self.name}{c}")
                        nc.sync.dma_start(
                            wt[:],
                            self.w_src[c].rearrange("p t co -> p (t co)"))
                        self.ws.append(wt)

            def emit(self, n):
                self.prefetch()
                emitted = 0
                while n > 0 and not self.done:
                    if self.i == 0:
                        self.cps = psum.tile([128, 512], F32, tag="cps",
                                             name=f"c_{self.name}{self.st}")
                    t, c = self.i // 2, self.i % 2
                    dy, dx = t // 3, t % 3
                    nc.tensor.matmul(
                        self.cps[:], self.ws[c][:, 128 * t:128 * (t + 1)],
                        xwin(c, 16 * self.st, 16, dy, dx),
                        start=(self.i == 0), stop=(self.i == 17))
                    self.i += 1
                    emitted += 1
                    n -= 1
                    if self.i == 18:
                        self.epi(self.st, self.cps)
                        self.i = 0
                        self.st += 1
                        self.done = self.st == 2
                return emitted

        def qkv_epi(cc):
            def epi(st, cps):
                b = bcol[:, cc:cc + 1]
                if cc < 2:
                    qv = q_st[cc][:].rearrange("p (c r) -> p r c", r=32)
                    nc.vector.tensor_scalar(
                        qv[:, 16 * st:16 * (st + 1), :],
                        cps[:].rearrange("p (r c) -> p r c", r=16),
                        b, SCALE, ADD, MULT)
                else:
                    dst = (k_st[cc - 2] if cc < 4 else v_st[cc - 4])
                    nc.vector.tensor_scalar(
                        dst[:, 512 * st:512 * (st + 1)], cps[:], b, None, ADD)
            return epi

        def xo_epi(cc):
            def epi(st, cps):
                osb = rel_pool.tile([128, 512], F32, tag="osb", name="osb")
                nc.vector.tensor_scalar(
                    osb[:], cps[:], bcol[:, 6 + cc:7 + cc], None, ADD)
                nc.sync.dma_start(
                    out_d[128 * cc:128 * (cc + 1),
                          512 * st:512 * (st + 1)], osb[:])
            return epi

        blocks = {
            "q0": Block(wq_d[0], qkv_epi(0), "q0"),  # ws pre-loaded above
        }
        blocks["q0"].ws = wq0
        blocks.update({
            "k0": Block(wq_d[2], qkv_epi(2), "k0"),
            "v0": Block(wq_d[4], qkv_epi(4), "v0"),
            "q1": Block(wq_d[1], qkv_epi(1), "q1"),
            "k1": Block(wq_d[3], qkv_epi(3), "k1"),
            "v1": Block(wq_d[5], qkv_epi(5), "v1"),
            "xo0": Block(wc_d[0], xo_epi(0), "xo0"),
            "xo1": Block(wc_d[1], xo_epi(1), "xo1"),
        })

        # ---------- rel tables + skew bounce (per (h, tab) step) ----------
        def rel_step(h, tab, copy_eng):
            qt, j4 = h // 4, h % 4
            rsb = rel_pool.tile([64, HW], BF16, tag="rsb", name="rsb")
            if h < 2:
                rl = psum.tile([128, HW], F32, tag="lps", name=f"rl{h}{tab}")
                views = [rl[:, 0:512], rl[:, 512:HW]]
            else:
                views = [psum.tile([128, 512], F32, tag="cps",
                                   name=f"r{h}{tab}{st}")[:]
                         for st in range(2)]
            for st in range(2):
                if tab == 0:
                    qrhs = q_st[qt][:, 512 * st:512 * (st + 1)]
                else:
                    qrhs = q_st[qt][:].rearrange(
                        "p (c r) -> p r c", r=32)[:, 16 * st:16 * (st + 1), :]
                rps = views[st]
                nc.tensor.matmul(rps, krc[tab][j4][:], qrhs,
                                 start=True, stop=True)
                if copy_eng is nc.scalar:
                    nc.scalar.activation(
                        rsb[0:63, 512 * st:512 * (st + 1)], rps[0:63, :],
                        mybir.ActivationFunctionType.Copy)
                else:
                    nc.vector.tensor_copy(
                        rsb[0:63, 512 * st:512 * (st + 1)], rps[0:63, :])
            dst = AP(skw[h][tab].tensor, 0, [[HW, 63], [1056, 32], [1, 32]])
            deng = nc.scalar if (h < 4 and tab == 1) else nc.sync
            deng.dma_start(
                dst, rsb[0:63, :].rearrange("p (a b) -> p a b", a=32))

        # ---------- v -> vT transpose steps ----------
        def vt_step(u, half):
            vps = psum.tile([128, 512], BF16, tag="cps", name=f"v{u}{half}")
            nc.tensor.transpose(
                vps[:, 0:128], v_st[half][:, 128 * u:128 * (u + 1)], i128[:])
            dst = vT_t[u][:].rearrange("p (h j) -> p h j", h=9)
            nc.vector.tensor_copy(
                dst[:, 4 * half:4 * (half + 1), 0:32],
                vps[:, 0:128].rearrange("p (h d) -> p h d", h=4))

        # ---------- filler queue (with pull-based forcing) ----------
        rel_done = set()
        vt_done = set()

        def do_rel(h, t):
            if (h, t) not in rel_done:
                rel_done.add((h, t))
                rel_step(h, t, nc.scalar if h < 2 else nc.vector)
                return 2
            return 0

        def do_vt(u, half):
            if (u, half) not in vt_done:
                vt_done.add((u, half))
                vt_step(u, half)
                return 1
            return 0

        fillers = (
            [("blk", "v0")]
            + [("vt", u, 0) for u in range(8)]
            + [("blk", "q1"), ("blk", "k1"), ("blk", "v1")]
            + [("vt", u, 1) for u in range(8)]
            + [("rel", h, t) for h in (4, 5, 6, 7) for t in range(2)]
            + [("blk", "xo0"), ("blk", "xo1")]
        )

        def emit_filler(budget):
            while budget > 0 and fillers:
                item = fillers[0]
                if item[0] == "blk":
                    blk = blocks[item[1]]
                    if blk.done:
                        fillers.pop(0)
                        continue
                    got = blk.emit(budget)
                    budget -= got
                    if blk.done:
                        fillers.pop(0)
                elif item[0] == "rel":
                    budget -= do_rel(*item[1:])
                    fillers.pop(0)
                else:
                    budget -= 2 * do_vt(*item[1:])
                    fillers.pop(0)

        def force_block(name):
            blk = blocks[name]
            if not blk.done:
                blk.emit(100)

        def ensure_stage_deps(h):
            if h >= 4:
                force_block("q1")
                force_block("k1")
            for t in range(2):
                do_rel(h, t)

        def ensure_vt(half):
            force_block("v1" if half else "v0")
            for u in range(8):
                do_vt(u, half)

        # prefetch ALL weight tiles up front: the sync queue is free now and
        # gets congested with skew-bounce traffic once the rel phase starts
        blocks["k0"].prefetch()
        blocks["v0"].prefetch()
        for name in ("q1", "k1", "v1", "xo0", "xo1"):
            blocks[name].prefetch()

        # ---------- pre-phase: q0 -> rel h0/h1 -> k0 -> staging ----------
        blocks["q0"].emit(36)

        # ---------- head staging ----------
        def stage_head(h):
            ensure_stage_deps(h)
            ceng = nc.vector
            meng = nc.vector if h < 4 else nc.gpsimd
            reng = nc.sync
            slot = h % 4
            lt, rtt = lt_r[slot], rt_r[slot]
            qt, j4 = h // 4, h % 4
            ceng.tensor_copy(lt[0:32, :],
                             k_st[qt][32 * j4:32 * j4 + 32, :])
            ceng.tensor_copy(rtt[0:32, :],
                             q_st[qt][32 * j4:32 * j4 + 32, :])
            skr = AP(skw[h][0].tensor, 31 * HW,
                     [[HW, 32], [32, 32], [1, 32]])
            reng.dma_start(
                rtt[32:64, :].rearrange("p (a b) -> p a b", a=32), skr)
            ah = xfer.tile([32, HW], BF16, tag="ah", name="ah", bufs=2)
            skr1 = AP(skw[h][1].tensor, 31 * HW,
                      [[HW, 32], [32, 32], [1, 32]])
            reng.dma_start(
                ah[:].rearrange("p (a b) -> p a b", a=32), skr1)
            meng.tensor_copy(
                rtt[64:96, :].rearrange("p (c r) -> p c r", c=32),
                ah[:].rearrange("p (r c) -> p c r", c=32))

        # rel h0/h1 right after q0 (latency chain, no filler spacing); the
        # k0 conv then overlaps the skew bounce DRAM round-trip
        for h in range(2):
            for t in range(2):
                do_rel(h, t)
        blocks["k0"].emit(36)
        for h in range(2):
            stage_head(h)
        # vT ones-columns + 1x1 weights; must be emitted BEFORE the first
        # vt_step evacuation below (the vinit DMA writes the whole tile)
        for u in range(8):
            nc.sync.dma_start(
                vT_t[u][:], AP(vinit_d.tensor, 0, [[0, 128], [1, 576]]))
        for i in range(2):
            nc.sync.dma_start(wattT_sb[i][:], wattT_d[i])
        # keep the PE on v0 while the h0/h1 skew bounce + staging reads
        # round-trip through DRAM (first logits otherwise stalls ~8us)
        emit_filler(40)

        # ---------- 1x1 conv on one m'-half (both cout tiles) ----------
        def final_conv(st):
            for ct in range(2):
                ops = psum.tile([128, 512], F32, tag="ap", name=f"o{ct}{st}")
                for kc in range(2):
                    nc.tensor.matmul(
                        ops[:],
                        wattT_sb[kc][:, 128 * ct:128 * (ct + 1)],
                        attn[kc][:, 512 * st:512 * (st + 1)],
                        start=(kc == 0), stop=(kc == 1))
                osb = rel_pool.tile([128, 512], F32, tag="osb", name="osb")
                nc.vector.tensor_scalar(
                    osb[:], ops[:], bcol[:, 8 + ct:9 + ct], None, ADD)
                eng = nc.sync if ct == 0 else nc.scalar
                eng.dma_start(
                    out_d[256 + 128 * ct:256 + 128 * (ct + 1),
                          512 * st:512 * (st + 1)],
                    osb[:])

        # ---------- head phase: flat (pair, u) stream, atts lag LAG groups
        # behind logits so the next pair's logits cross the boundary ahead
        # of the previous pair's PSUM evacuation (keeps ScalarE's exp
        # stream fed through pair transitions) ----------
        LAG = 3
        es = {}
        aps = {}

        def emit_logits(p, u):
            for j in range(2):
                s = (2 * p + j) % 4
                lt, rtt = lt_r[s], rt_r[s]
                lps = psum.tile([128, HW], F32, tag="lps",
                                name=f"l{p}_{u}_{j}")
                for mh in range(2):
                    nc.tensor.matmul(
                        lps[:, 512 * mh:512 * (mh + 1)],
                        lt[:, 128 * u:128 * (u + 1)],
                        rtt[:, 512 * mh:512 * (mh + 1)],
                        start=True, stop=True)
                e = e_pool.tile([128, HW], BF16, tag="E", name=f"e{p}{u}{j}")
                nc.scalar.activation(e[:], lps[:], Exp)
                es[(p, u, j)] = e

        def pair_end(p):
            # evacuate att + sums via 32x32 block transposes, one m'-half
            # at a time so the last pair's normalize + 1x1 conv pipeline
            h0, h1 = 2 * p, 2 * p + 1
            g = p // 2
            po0, po1 = 32 * (h0 % 4), 32 * (h1 % 4)
            ap0, ap1 = aps.pop(p)
            for mh, apx in ((0, ap0), (1, ap1)):
                c0, c1 = 512 * mh, 512 * (mh + 1)
                nc.vector.transpose(att_un[g][po0:po0 + 32, c0:c1],
                                    apx[0:32, :])
                nc.vector.transpose(smap[g][po0:po0 + 32, c0:c1],
                                    apx[32:64, :])
                nc.vector.transpose(att_un[g][po1:po1 + 32, c0:c1],
                                    apx[64:96, :])
                nc.vector.transpose(smap[g][po1:po1 + 32, c0:c1],
                                    apx[96:128, :])
                if p % 2 == 1:
                    nc.vector.reciprocal_approx_fast(
                        rmap[g][:, c0:c1], smap[g][:, c0:c1])
                    nc.vector.tensor_tensor(
                        attn[g][:, c0:c1], att_un[g][:, c0:c1],
                        rmap[g][:, c0:c1], MULT)
                    if p == 3:
                        final_conv(mh)

        def emit_att(p, u):
            h0, h1 = 2 * p, 2 * p + 1
            if u == 0:
                force_block("v0" if h0 < 4 else "v1")
                aps[p] = (
                    psum.tile([128, 512], F32, tag="ap", name=f"ap0_{p}"),
                    psum.tile([128, 512], F32, tag="ap", name=f"ap1_{p}"))
            ap0, ap1 = aps[p]
            do_vt(u, h0 // 4)
            e0, e1 = es.pop((p, u, 0)), es.pop((p, u, 1))
            for mh, apx in ((0, ap0), (1, ap1)):
                nc.tensor.matmul(
                    apx[0:64, :],
                    vT_t[u][:, 64 * h0:64 * h0 + 64],
                    e0[:, 512 * mh:512 * (mh + 1)],
                    start=(u == 0), stop=(u == 7),
                    tile_position=(0, 0), skip_group_check=True)
                nc.tensor.matmul(
                    apx[64:128, :],
                    vT_t[u][:, 64 * h1:64 * h1 + 64],
                    e1[:, 512 * mh:512 * (mh + 1)],
                    start=(u == 0), stop=(u == 7),
                    tile_position=(0, 64), skip_group_check=True)
            if u == 7:
                pair_end(p)

        groups = [(p, u) for p in range(4) for u in range(8)]
        for i in range(len(groups) + LAG):
            if i < len(groups):
                p, u = groups[i]
                emit_logits(p, u)
                if u == 2 and p < 3:
                    stage_head(2 * p + 2)
                    stage_head(2 * p + 3)
            emit_filler(FILL_PER_U)
            if i >= LAG:
                emit_att(*groups[i - LAG])

        # drain any remaining filler work
        emit_filler(10 ** 6)


def _host_inputs(x, w_conv, b_conv, w_qkv, b_qkv, w_att, b_att,
                 key_rel_w, key_rel_h):
    """Build per-core input maps (host-side layout prep only)."""
    import ml_dtypes
    bf16 = ml_dtypes.bfloat16
    x = np.asarray(x, dtype=np.float32)

    def wT(w, nt):
        # (co, ci, 3, 3) -> (cout_tile, ci_half, ci_local, tap, co_local)
        w = np.asarray(w, dtype=np.float32).reshape(nt, 128, 2, 128, 9)
        return np.ascontiguousarray(w.transpose(0, 2, 3, 4, 1)).astype(bf16)

    wqkvT = wT(w_qkv, 6)
    wconvT = wT(w_conv, 2)
    wattT = np.ascontiguousarray(
        np.asarray(w_att, dtype=np.float32)[:, :, 0, 0].T.reshape(
            2, 128, 256)).astype(bf16)
    bias_cols = np.zeros((128, 10), np.float32)
    bias_cols[:, 0:6] = np.asarray(b_qkv, np.float32).reshape(6, 128).T
    bias_cols[:, 6:8] = np.asarray(b_conv, np.float32).reshape(2, 128).T
    bias_cols[:, 8:10] = np.asarray(b_att, np.float32).reshape(2, 128).T
    n = np.arange(HW)
    ltc = np.zeros((96, HW), np.float32)
    ltc[0:32] = np.arange(32)[:, None] == (n % 32)[None, :]   # I_w
    ltc[32:64] = np.arange(32)[:, None] == (n // 32)[None, :]  # I_h
    I128 = np.eye(128, dtype=np.float32).astype(bf16)
    vinit = np.tile(np.concatenate([
        np.zeros(32, np.float32), np.ones(32, np.float32)]), 9).astype(bf16)
    krcb = np.zeros((2, 4, 128, 128), np.float32)
    for t, kr in ((0, key_rel_w), (1, key_rel_h)):
        krT = np.asarray(kr, np.float32).T  # (32, 63)
        for a in range(4):
            krcb[t, a, 32 * a:32 * (a + 1), 0:63] = krT
    shared = {
        "wqkvT": wqkvT, "wconvT": wconvT, "wattT": wattT,
        "bias_cols": bias_cols,
        "krcb": krcb.astype(bf16),
        "ltc": ltc.astype(bf16),
        "rtz": np.zeros((32, HW), bf16),
        "I128": I128,
        "vinit": vinit,
    }
    xp = np.zeros((B, 2, 128, 34, 34), np.float32)
    xp[:, :, :, 1:33, 1:33] = x.reshape(B, 2, 128, 32, 32)
    xp = xp.reshape(B, 2, 128, 34 * 34).astype(bf16)
    return [dict(shared, x=xp[i]) for i in range(B)]


def get_nc():
    if "nc" not in _cached:
        _cached["nc"] = build_bass()
    return _cached["nc"]


def kernel(x, w_conv, b_conv, w_qkv, b_qkv, w_att, b_att,
           key_rel_w, key_rel_h):
    from concourse.bass_utils import run_bass_kernel_spmd
    nc = get_nc()
    in_maps = _host_inputs(x, w_conv, b_conv, w_qkv, b_qkv, w_att, b_att,
                           key_rel_w, key_rel_h)
    res = run_bass_kernel_spmd(nc, in_maps, list(range(N_CORES)))
    out = np.stack([res.results[i]["out"].reshape(COUT, H, W)
                    for i in range(B)])
    return out
